# revision 9
# baseline (speedup 1.0000x reference)
"""Multi-head attention (B=4, S=2048, D=1024, H=16, Dh=64) on 8 NeuronCores.

Sharding: core c handles batch b=c//2 and head-group g=c%2 (8 heads).
wq/wk/wv column-parallel, wo row-parallel; host sums the two partial
wo-products per batch and adds bo.

Per-core kernel (matmul operands in bf16 = 1 cyc/row PE streaming + FWL;
accumulation always f32 in PSUM; softmax normalization in f32):
  phase 1: Q^T,K^T [512,2048] and V [2048, 8x(64+ones)] projections
  phase 2: causal flash attention per (head, 512-wide q chunk):
           scores_T[sk,sq] = K^T_tile.T @ Q^T_chunk  (K=64 contraction)
           attn_T = exp(0.125*scores + causal_mask)   (ACT, bf16 out)
           outT_aug[65,sq] += [v|1].T @ attn_T        (PSUM accumulate)
           normalize: bcast denom row via PE outer product, then
           reciprocal_approx_fast + multiply in f32 on PSUM
  phase 3: out_partial[s,1024] = attn_outT.T @ woT   (K=512 per core)
"""

import sys

sys.path.insert(0, "/opt/trn_rl_repo")

import ml_dtypes
import numpy as np

import concourse.bass as bass  # noqa: F401
import concourse.bacc as bacc
import concourse.tile as tile
import concourse.mybir as mybir
from concourse.bass_utils import run_bass_kernel_spmd

F32 = mybir.dt.float32
F32R = mybir.dt.float32r
BF16 = mybir.dt.bfloat16
AF = mybir.ActivationFunctionType
BF = ml_dtypes.bfloat16

B, S, D = 4, 2048, 1024
H, DH = 16, 64
HG = 8  # heads per core
DG = HG * DH  # 512 out-dims per core
NEG = -1.0e9

_PROGRAM = None
LAST_RESULTS = None  # for test.py introspection


def _build_program():
    nc = bacc.Bacc("TRN2", target_bir_lowering=False, debug=False)

    xq_t = nc.dram_tensor("xq_t", [D, S], BF16, kind="ExternalInput")
    xk_t = nc.dram_tensor("xk_t", [D, S], BF16, kind="ExternalInput")
    xv_t = nc.dram_tensor("xv_t", [D, S], BF16, kind="ExternalInput")
    wq_t = nc.dram_tensor("wq_t", [D, DG], BF16, kind="ExternalInput")
    wk_t = nc.dram_tensor("wk_t", [D, DG], BF16, kind="ExternalInput")
    wv_t = nc.dram_tensor("wv_t", [D, DG], BF16, kind="ExternalInput")
    wo_t = nc.dram_tensor("wo_t", [DG, D], BF16, kind="ExternalInput")
    bq_c = nc.dram_tensor("bq_c", [128, 4], F32, kind="ExternalInput")
    bk_c = nc.dram_tensor("bk_c", [128, 4], F32, kind="ExternalInput")
    bv_r = nc.dram_tensor("bv_r", [1, DG], BF16, kind="ExternalInput")
    ones_b = nc.dram_tensor("ones_b", [1, 128], BF16, kind="ExternalInput")
    ones_f = nc.dram_tensor("ones_f", [1, 128], F32R, kind="ExternalInput")
    ones8 = nc.dram_tensor("ones8", [128, 8], BF16, kind="ExternalInput")
    maskadd = nc.dram_tensor("maskadd", [128, 128], F32, kind="ExternalInput")
    outs = [
        nc.dram_tensor(f"out{c}", [S, D], F32, kind="ExternalOutput")
        for c in range(4)
    ]

    with tile.TileContext(nc) as tc:
        with (
            nc.allow_low_precision(reason="bf16 attention pipeline"),
            tc.tile_pool(name="persist", bufs=1) as pers,
        ):
            # ---- persistent tiles ----
            qT = [pers.tile([128, S], BF16, name=f"qT{i}") for i in range(4)]
            kT = [pers.tile([128, S], BF16, name=f"kT{i}") for i in range(4)]
            # v tiles: [128 s, 8 heads x (64 v + 1 ones)]
            vt = [pers.tile([128, HG * 65], BF16, name=f"v{i}") for i in range(16)]
            aout = [pers.tile([128, S], BF16, name=f"ao{i}") for i in range(4)]
            mask_sb = pers.tile([128, 128], F32, name="mask")
            ones_bf = pers.tile([1, 128], BF16, name="ones_bf")
            ones_fr = pers.tile([65, 128], F32R, name="ones_fr")
            bq_sb = pers.tile([128, 4], F32, name="bq")
            bk_sb = pers.tile([128, 4], F32, name="bk")
            bv_sb = pers.tile([1, DG], BF16, name="bv")

            nc.sync.dma_start(out=mask_sb[:], in_=maskadd[:])
            nc.sync.dma_start(out=ones_bf[:], in_=ones_b[:])
            nc.sync.dma_start(out=ones_fr[64:65, :], in_=ones_f[:])
            nc.sync.dma_start(out=bq_sb[:], in_=bq_c[:])
            nc.sync.dma_start(out=bk_sb[:], in_=bk_c[:])
            nc.sync.dma_start(out=bv_sb[:], in_=bv_r[:])

            # ---- unified psum pool: "ps" 4 banks, "po" 3, "pb" 1 ----
            pp = tc.alloc_tile_pool(name="pp", bufs=4, space="PSUM")
            # ---- phase 1: projections ----
            with (
                tc.tile_pool(name="wbig", bufs=2) as wp,
                tc.tile_pool(name="xbig", bufs=3) as xp,
            ):
                # Q^T and K^T: out[dq, s] = wT.T @ xT
                for x_d, w_d, b_sb, dst in (
                    (xq_t, wq_t, bq_sb, qT),
                    (xk_t, wk_t, bk_sb, kT),
                ):
                    # whole weight block in one DMA: [128, 8k x 512dq]
                    w_big = wp.tile([128, 8 * DG], BF16, tag="wb", name="w_big")
                    nc.sync.dma_start(
                        out=w_big[:].rearrange("p (k d) -> p k d", k=8),
                        in_=w_d[:].rearrange("(k p) d -> p k d", p=128),
                    )
                    for n in range(4):
                        x_big = xp.tile([128, 8 * 512], BF16, tag="xb", name="x_big")
                        nc.sync.dma_start(
                            out=x_big[:].rearrange("p (k s) -> p k s", k=8),
                            in_=x_d[:, n * 512 : (n + 1) * 512].rearrange(
                                "(k p) s -> p k s", p=128
                            ),
                        )
                        for m in range(4):
                            ps = pp.tile([128, 512], F32, tag="ps", name="ps1")
                            for k8 in range(8):
                                nc.tensor.matmul(
                                    ps[:],
                                    w_big[
                                        :,
                                        k8 * DG + m * 128 : k8 * DG + (m + 1) * 128,
                                    ],
                                    x_big[:, k8 * 512 : (k8 + 1) * 512],
                                    start=(k8 == 0),
                                    stop=(k8 == 7),
                                )
                            nc.scalar.activation(
                                dst[m][:, n * 512 : (n + 1) * 512],
                                ps[:],
                                AF.Identity,
                                bias=b_sb[:, m : m + 1],
                            )

                # V: out[s_tile, dv] = xvT_tile.T @ wvT (+ ones-row bias)
                wv_big = wp.tile([128, 8 * DG], BF16, tag="wb", name="wv_big")
                nc.sync.dma_start(
                    out=wv_big[:].rearrange("p (k d) -> p k d", k=8),
                    in_=wv_t[:].rearrange("(k p) d -> p k d", p=128),
                )
                for s in range(16):
                    xv_big = xp.tile([128, 8 * 128], BF16, tag="xvb", name="xv_big")
                    nc.sync.dma_start(
                        out=xv_big[:].rearrange("p (k s2) -> p k s2", k=8),
                        in_=xv_t[:, s * 128 : (s + 1) * 128].rearrange(
                            "(k p) s2 -> p k s2", p=128
                        ),
                    )
                    ps = pp.tile([128, DG], F32, tag="ps", name="psv")
                    for k8 in range(8):
                        nc.tensor.matmul(
                            ps[:],
                            xv_big[:, k8 * 128 : (k8 + 1) * 128],
                            wv_big[:, k8 * DG : (k8 + 1) * DG],
                            start=(k8 == 0),
                            stop=False,
                        )
                    nc.tensor.matmul(
                        ps[:], ones_bf[:], bv_sb[:], start=False, stop=True
                    )
                    v3 = vt[s].rearrange("p (h x) -> p h x", x=65)
                    nc.vector.tensor_copy(
                        v3[:, :, 0:64],
                        ps[:].rearrange("p (h d) -> p h d", d=64),
                    )
                    nc.sync.dma_start(out=v3[:, :, 64:65], in_=ones8[:].unsqueeze(2))

            # ---- phase 2: causal attention, with Wo chunk bursts ----
            with (
                tc.tile_pool(name="at", bufs=3) as ap_,
                tc.tile_pool(name="sm", bufs=4) as sm,
                tc.tile_pool(name="wo", bufs=4) as wop,
                tc.tile_pool(name="ob", bufs=4) as obp,
            ):
                wo_sb = [
                    wop.tile([128, D], BF16, tag="wo", name=f"wo{c}")
                    for c in range(4)
                ]
                for c in range(4):
                    nc.sync.dma_start(
                        out=wo_sb[c][:], in_=wo_t[c * 128 : (c + 1) * 128, :]
                    )
                for h in range(HG):
                    ht, hp = h // 2, (h % 2) * 64
                    for j in range(4):
                        nsk = 4 * j + 4
                        ps_o = pp.tile([65, 512], F32, tag="po", bufs=3, name="ps_o")
                        for i in range(nsk):
                            koff = i - 4 * j
                            c0 = max(0, koff * 128) if koff >= 0 else 0
                            ps_s = pp.tile([128, 512], F32, tag="ps", name="ps_s")
                            nc.tensor.matmul(
                                ps_s[:, c0:512],
                                kT[ht][hp : hp + 64, i * 128 : (i + 1) * 128],
                                qT[ht][hp : hp + 64, j * 512 + c0 : (j + 1) * 512],
                                start=True,
                                stop=True,
                            )
                            if koff >= 0:
                                # mask diagonal 128x128 block (pre-exp add)
                                nc.vector.tensor_add(
                                    ps_s[:, c0 : c0 + 128],
                                    ps_s[:, c0 : c0 + 128],
                                    mask_sb[:],
                                )
                            at = ap_.tile([128, 512], BF16, tag="at", name="at")
                            nc.scalar.activation(
                                at[:, c0:512], ps_s[:, c0:512], AF.Exp, scale=0.125
                            )
                            nc.tensor.matmul(
                                ps_o[:, c0:512],
                                vt[i][:, h * 65 : h * 65 + 65],
                                at[:, c0:512],
                                start=(i == 0),
                                stop=(i == nsk - 1),
                            )
                        # normalize: bcast raw denom row via PE outer product,
                        # reciprocal_approx_fast on [64,512] PSUM (proven),
                        # multiply + copy on DVE
                        den = sm.tile([65, 512], F32R, tag="den", name="den")
                        nc.vector.tensor_copy(den[64:65, :], ps_o[64:65, :])
                        ps_bc = pp.tile([64, 512], F32, tag="pb", bufs=1, name="ps_bc")
                        nc.tensor.matmul(
                            ps_bc[:],
                            ones_fr[64:65, 0:64],
                            den[64:65, :],
                            start=True,
                            stop=True,
                        )
                        rb = sm.tile([64, 512], F32, tag="rb", name="rb")
                        nc.vector.reciprocal_approx_fast(out=rb[:], in_=ps_bc[:])
                        nc.vector.tensor_mul(ps_o[0:64, :], ps_o[0:64, :], rb[:])
                        dst = aout[ht][hp : hp + 64, j * 512 : (j + 1) * 512]
                        if hp == 0:
                            nc.vector.tensor_copy(dst, ps_o[0:64, :])
                        else:
                            tmp = sm.tile([64, 512], BF16, tag="tmp", name="tmp")
                            nc.vector.tensor_copy(tmp[:], ps_o[0:64, :])
                            nc.sync.dma_start(out=dst, in_=tmp[:])
                    if hp != 0:
                        # dense full-array Wo burst for completed head pair:
                        # keeps the PE HAM un-throttled and retires phase-3
                        # work during the ACT-paced attention
                        c = ht
                        for s in range(16):
                            for n2 in range(2):
                                psw = pp.tile([128, 512], F32, tag="ps", name="psw")
                                nc.tensor.matmul(
                                    psw[:],
                                    aout[c][:, s * 128 : (s + 1) * 128],
                                    wo_sb[c][:, n2 * 512 : (n2 + 1) * 512],
                                    start=True,
                                    stop=True,
                                )
                                ob = obp.tile([128, 512], F32, tag="ob", name="ob")
                                nc.vector.tensor_copy(ob[:], psw[:])
                                nc.sync.dma_start(
                                    out=outs[c][
                                        s * 128 : (s + 1) * 128,
                                        n2 * 512 : (n2 + 1) * 512,
                                    ],
                                    in_=ob[:],
                                )

            pp.release()

    nc.compile()
    return nc


def _make_in_maps(query, key, value, wq, bq, wk, bk, wv, bv, wo):
    f32 = np.float32
    ones_b = np.ones((1, 128), BF)
    ones_f = np.ones((1, 128), np.float32)
    ones8 = np.ones((128, 8), BF)
    # causal mask add-block in scores_T layout: rows=sk_local, cols=sq_local;
    # valid iff sq_local >= sk_local
    maskadd = np.where(
        np.triu(np.ones((128, 128), bool)), f32(0), f32(NEG)
    ).astype(f32)

    wqT = np.asarray(wq, f32).T.astype(BF)  # [D, D] (d, dq)
    wkT = np.asarray(wk, f32).T.astype(BF)
    wvT = np.asarray(wv, f32).T.astype(BF)
    woT = np.asarray(wo, f32).T.astype(BF)  # [dv, D]

    in_maps = []
    for c in range(8):
        b, g = c // 2, c % 2
        sl = slice(g * DG, (g + 1) * DG)
        in_maps.append(
            {
                "xq_t": np.ascontiguousarray(np.asarray(query[b], f32).T.astype(BF)),
                "xk_t": np.ascontiguousarray(np.asarray(key[b], f32).T.astype(BF)),
                "xv_t": np.ascontiguousarray(np.asarray(value[b], f32).T.astype(BF)),
                "wq_t": np.ascontiguousarray(wqT[:, sl]),
                "wk_t": np.ascontiguousarray(wkT[:, sl]),
                "wv_t": np.ascontiguousarray(wvT[:, sl]),
                "wo_t": np.ascontiguousarray(woT[sl, :]),
                "bq_c": np.ascontiguousarray(
                    np.asarray(bq, f32)[sl].reshape(4, 128).T
                ),
                "bk_c": np.ascontiguousarray(
                    np.asarray(bk, f32)[sl].reshape(4, 128).T
                ),
                "bv_r": np.asarray(bv, f32)[sl].reshape(1, DG).astype(BF),
                "ones_b": ones_b,
                "ones_f": ones_f,
                "ones8": ones8,
                "maskadd": maskadd,
            }
        )
    return in_maps


def kernel(query, key, value, mask, wq, bq, wk, bk, wv, bv, wo, bo):
    global _PROGRAM, LAST_RESULTS
    if _PROGRAM is None:
        _PROGRAM = _build_program()
    nc = _PROGRAM
    in_maps = _make_in_maps(query, key, value, wq, bq, wk, bk, wv, bv, wo)

    res = run_bass_kernel_spmd(nc, in_maps, core_ids=list(range(8)))
    LAST_RESULTS = res

    f32 = np.float32
    out = np.empty((B, S, D), f32)
    for b in range(B):
        acc = None
        for r in (res.results[2 * b], res.results[2 * b + 1]):
            for c in range(4):
                acc = r[f"out{c}"] if acc is None else acc + r[f"out{c}"]
        out[b] = acc
    out += np.asarray(bo, f32)[None, None, :]
    return out
